# revision 14
# baseline (speedup 1.0000x reference)
"""AdaIN statistics kernel for TRN2, SPMD across 8 NeuronCores.

Input : f_vol [32, 512, 64, 64] f32
Output: [32, 1024] f32 = concat([mean over (h,w), unbiased std over (h,w)], axis=-1)

Sharding: data-parallel over batch — each of the 8 cores handles 4 batches
([4, 512, 64, 64] shard, 32 MiB). No collectives; the host concatenates the
8 per-core [4, 1024] outputs.

Per core: view the shard as 2048 rows (b*512+c) x 4096 spatial elems.
16 tiles of [128 rows, 4096]. Pipeline (raw Bass, manual semaphores —
Tile's scheduler emits 2 sync-waits on slot-reuse DMAs, which this
compiler's static-DMA encoding cannot hold):
  SP/ACT/POOL : input DMAs round-robined over 3 queue rings so the 16
                SDMA engines interleave packets from multiple rings
                (single-ring measured ~16 GB/s per engine; multi-ring
                lets each engine hide per-packet HBM latency)
  DVE         : one 3D-AP bn_stats ([128, 8, 512] -> [128, 8, 6]) +
                bn_aggr per tile -> (mean, biased var)
  ACT         : mean copy + sqrt(var * N/(N-1)), then tiny output DMAs

DMA completion is not FIFO across in-flight transfers, so each input ring
slot / res slot gets its own DMA-completion semaphore (mirrors Tile's
DMASW lanes). Every cross-instruction data edge is covered by an explicit
semaphore observation so the CoreSim race detector can verify the design.
"""

from contextlib import ExitStack

import numpy as np

B, C, H, W = 32, 512, 64, 64
N_CORES = 8
B_LOCAL = B // N_CORES  # 4
N = H * W  # 4096
P = 128
ROWS = B_LOCAL * C  # 2048
NTILES = ROWS // P  # 16
TPB = C // P  # tiles (channel blocks) per batch = 4
G = N // 512  # bn_stats groups per row = 8
NBUF = 8  # input ring slots (8 x 16 KiB/partition)
NSMALL = 4  # stats/mv/res ring slots

_CACHE = {}


def _bn_stats_3d(vector, out_ap, in_ap):
    """bn_stats with a multi-group access pattern ([P, G, 512] -> [P, G, 6]).

    The bass wrapper caps total free size at 512; the HW restriction is on
    the innermost (group) dim only — emit InstBNStats directly.
    """
    from concourse import mybir

    return vector.add_instruction(
        mybir.InstBNStats(
            name=vector.bass.get_next_instruction_name(),
            ins=[vector.lower_ap(in_ap, opt=False)],
            outs=[vector.lower_ap(out_ap, opt=False)],
        )
    )


def _build():
    import concourse.bass as bass
    from concourse import mybir

    nc = bass.Bass()
    x_ext = nc.declare_dram_parameter(
        "f_vol", [B_LOCAL, C, H, W], mybir.dt.float32, isOutput=False
    )
    out_ext = nc.declare_dram_parameter(
        "out", [B_LOCAL, 2 * C], mybir.dt.float32, isOutput=True
    )

    x = x_ext.ap().rearrange("b c h w -> (b c) (h w)")  # [2048, 4096]
    o3 = out_ext.ap().rearrange("b (s c) -> b s c", s=2)  # [4, 2, 512]

    # input tile t is issued by this engine's queue ring (3-way round-robin)
    ISSUER = [t % 3 for t in range(NTILES)]  # 0=sync, 1=scalar, 2=gpsimd

    with ExitStack() as ctx:
        block = ctx.enter_context(nc.Block())
        dma_in = [ctx.enter_context(nc.semaphore(f"dma_in{t}")) for t in range(NTILES)]
        dma_out = [
            ctx.enter_context(nc.semaphore(f"dma_out{k}")) for k in range(NSMALL)
        ]
        dve_stats = ctx.enter_context(nc.semaphore("dve_stats"))  # +1 per bn_stats
        mv_ready = ctx.enter_context(nc.semaphore("mv_ready"))  # +1 per bn_aggr
        act_done = ctx.enter_context(nc.semaphore("act_done"))  # +2 per tile (ACT)
        xt = ctx.enter_context(nc.sbuf_tensor("xt", [P, NBUF, N], mybir.dt.float32))
        stats = ctx.enter_context(
            nc.sbuf_tensor("stats", [P, NSMALL, G, 6], mybir.dt.float32)
        )
        mv = ctx.enter_context(nc.sbuf_tensor("mv", [P, NSMALL, 2], mybir.dt.float32))
        res = ctx.enter_context(
            nc.sbuf_tensor("res", [P, NSMALL, 2], mybir.dt.float32)
        )

        def issue_input(eng, t):
            k = t % NBUF
            if t >= NBUF:
                # slot k free once DVE's bn_stats of tile t-NBUF retired,
                # and the slot's previous DMA completion has been observed
                eng.wait_ge(dve_stats, G * (t - NBUF + 1))
                eng.wait_ge(dma_in[t - NBUF], 16)
            eng.dma_start(out=xt[:, k, :], in_=x[t * P : (t + 1) * P, :]).then_inc(
                dma_in[t], 16
            )

        @block.sync
        def _(sync):
            for t in range(NTILES):
                if ISSUER[t] == 0:
                    issue_input(sync, t)
            # keep the NEFF alive until every output DMA has landed
            for s in range(NSMALL):
                sync.wait_ge(dma_out[s], 32 * (NTILES // NSMALL))

        @block.gpsimd
        def _(gpsimd):
            for t in range(NTILES):
                if ISSUER[t] == 2:
                    issue_input(gpsimd, t)

        @block.vector
        def _(vector):
            for t in range(NTILES):
                k = t % NBUF
                s = t % NSMALL
                vector.wait_ge(dma_in[t], 16)
                if t >= NSMALL:
                    # stats slot WAR: bn_aggr of tile t-NSMALL has read it
                    vector.wait_ge(mv_ready, t - NSMALL + 1)
                for g in range(G):
                    vector.bn_stats(
                        out=stats[:, s, g, :],
                        in_=xt[:, k, g * 512 : (g + 1) * 512],
                    ).then_inc(dve_stats, 1)
                if t >= NSMALL:
                    # mv slot WAR: ACT of tile t-NSMALL has read it
                    vector.wait_ge(act_done, 2 * (t - NSMALL) + 2)
                # stats RAW: the bn_stats writes of THIS tile retired
                vector.wait_ge(dve_stats, G * (t + 1))
                vector.bn_aggr(out=mv[:, s, :], in_=stats[:, s, :, :]).then_inc(
                    mv_ready, 1
                )

        @block.scalar
        def _(scalar):
            # scalar's own input tiles are issued with a +4 lookahead so the
            # issue never sits behind this engine's mv_ready stalls
            for u in range(min(4, NTILES)):
                if ISSUER[u] == 1:
                    issue_input(scalar, u)
            for t in range(NTILES):
                s = t % NSMALL
                b, cb = divmod(t, TPB)
                u = t + 4
                if u < NTILES and ISSUER[u] == 1:
                    issue_input(scalar, u)
                scalar.wait_ge(mv_ready, t + 1)
                if t >= NSMALL:
                    # res slot reused: out-DMAs of tile t-NSMALL must be done
                    scalar.wait_ge(dma_out[s], 32 * (t // NSMALL))
                scalar.copy(out=res[:, s, 0:1], in_=mv[:, s, 0:1]).then_inc(
                    act_done, 1
                )
                scalar.activation(
                    out=res[:, s, 1:2],
                    in_=mv[:, s, 1:2],
                    func=mybir.ActivationFunctionType.Sqrt,
                    scale=float(N) / (N - 1),
                ).then_inc(act_done, 1)
                # res RAW: both ACT writes retired before the DMAs read them
                scalar.wait_ge(act_done, 2 * t + 2)
                # means -> out[b, cb*128 : (cb+1)*128], stds -> out[b, 512+...]
                scalar.dma_start(
                    out=o3[b, 0, cb * P : (cb + 1) * P], in_=res[:, s, 0:1]
                ).then_inc(dma_out[s], 16)
                scalar.dma_start(
                    out=o3[b, 1, cb * P : (cb + 1) * P], in_=res[:, s, 1:2]
                ).then_inc(dma_out[s], 16)

    return nc


def kernel(f_vol: np.ndarray) -> np.ndarray:
    from concourse.bass_utils import run_bass_kernel_spmd

    if "nc" not in _CACHE:
        _CACHE["nc"] = _build()
    nc = _CACHE["nc"]

    f_vol = np.ascontiguousarray(f_vol, dtype=np.float32)
    in_maps = [
        {"f_vol": f_vol[i * B_LOCAL : (i + 1) * B_LOCAL]} for i in range(N_CORES)
    ]
    res = run_bass_kernel_spmd(nc, in_maps, core_ids=list(range(N_CORES)))
    return np.concatenate([res.results[i]["out"] for i in range(N_CORES)], axis=0)


# revision 18
# speedup vs baseline: 1.4360x; 1.4360x over previous
"""AdaIN statistics kernel for TRN2, SPMD across 8 NeuronCores.

Input : f_vol [32, 512, 64, 64] f32
Output: [32, 1024] f32 = concat([mean over (h,w), unbiased std over (h,w)], axis=-1)

Sharding: data-parallel over batch — each of the 8 cores handles 4 batches
([4, 512, 64, 64] shard, 32 MiB). No collectives; the host concatenates the
8 per-core [4, 1024] outputs.

Per core: view the shard as 2048 rows (b*512+c) x 4096 spatial elems.
The shard is streamed in SLABS: a slab with m rows/partition loads
128*m consecutive rows, partition p holding rows base+p*m .. +m (so each
partition's DRAM chunk is m*16 KiB contiguous -> m*16 KiB DMA descriptors;
measured per-SDMA-engine rate is descriptor-size-bound: ~16 GB/s at 16 KiB,
~0.35 us fixed per packet). Big slabs mid-stream maximize bandwidth; small
slabs at the end keep the compute tail short.

Raw Bass with manual semaphores (Tile's scheduler emits 2 sync-waits on
slot-reuse DMAs, which this compiler's static-DMA encoding cannot hold):
  SP  : input slab DMAs (ring of 3 x 64 KiB/partition SBUF slots)
  DVE : 8 bn_stats per row + bn_aggr per row -> (mean, biased var)
  ACT : mean copy + sqrt(var * N/(N-1)) per row, one output DMA per slab

DMA completion is not FIFO across in-flight transfers, so each slab gets
its own single-use DMA-completion semaphore. Every cross-instruction data
edge is covered by an explicit semaphore observation so the CoreSim race
detector can verify the design.
"""

from contextlib import ExitStack

import numpy as np

B, C, H, W = 32, 512, 64, 64
N_CORES = 8
B_LOCAL = B // N_CORES  # 4
N = H * W  # 4096
P = 128
ROWS = B_LOCAL * C  # 2048
G = N // 512  # bn_stats groups per row = 8

# rows-per-partition per slab; each slab (128*m rows) must stay inside one
# batch (128*m | 512 alignment held by construction below)
SLABS = [4, 4, 4, 2, 1, 1]
assert sum(SLABS) * P == ROWS
MMAX = max(SLABS)
NBUF = 3  # input slab ring slots (3 x MMAX*16 KiB/partition)
NSMALL = 2  # stats/mv/res ring slots

_CACHE = {}


def _build():
    import concourse.bass as bass
    from concourse import mybir

    nc = bass.Bass()
    x_ext = nc.declare_dram_parameter(
        "f_vol", [B_LOCAL, C, H, W], mybir.dt.float32, isOutput=False
    )
    out_ext = nc.declare_dram_parameter(
        "out", [B_LOCAL, 2 * C], mybir.dt.float32, isOutput=True
    )

    x = x_ext.ap().rearrange("b c h w -> (b c) (h w)")  # [2048, 4096]

    nslabs = len(SLABS)
    base_rows = [P * sum(SLABS[:j]) for j in range(nslabs)]
    for j, m in enumerate(SLABS):
        assert base_rows[j] % (P * m) == 0 or True
        assert (base_rows[j] % C) + P * m <= C, f"slab {j} crosses a batch"
    cum_rows = [sum(SLABS[: j + 1]) for j in range(nslabs)]  # rows/partition
    cum_stats = [G * c for c in cum_rows]  # bn_stats count after slab j
    cum_aggr = cum_rows  # bn_aggr count after slab j
    cum_act = [2 * c for c in cum_rows]  # ACT op count after slab j

    with ExitStack() as ctx:
        block = ctx.enter_context(nc.Block())
        dma_in = [
            ctx.enter_context(nc.semaphore(f"dma_in{j}")) for j in range(nslabs)
        ]
        dma_out = [
            ctx.enter_context(nc.semaphore(f"dma_out{s}")) for s in range(NSMALL)
        ]
        dve_stats = ctx.enter_context(nc.semaphore("dve_stats"))
        mv_ready = ctx.enter_context(nc.semaphore("mv_ready"))
        act_done = ctx.enter_context(nc.semaphore("act_done"))
        xt = ctx.enter_context(
            nc.sbuf_tensor("xt", [P, NBUF, MMAX * N], mybir.dt.float32)
        )
        stats = ctx.enter_context(
            nc.sbuf_tensor("stats", [P, NSMALL, MMAX, G, 6], mybir.dt.float32)
        )
        mv = ctx.enter_context(
            nc.sbuf_tensor("mv", [P, NSMALL, MMAX, 2], mybir.dt.float32)
        )
        res = ctx.enter_context(
            nc.sbuf_tensor("res", [P, NSMALL, 2, MMAX], mybir.dt.float32)
        )

        # out-DMA count per slab: m==1 needs two DMAs (per-stat contiguous
        # runs), otherwise one 3D-AP DMA covers both stats
        out_incs = [32 if m == 1 else 16 for m in SLABS]
        out_total = {s: 0 for s in range(NSMALL)}
        out_after = []  # dma_out[j % NSMALL] value after slab j's DMAs land
        for j, m in enumerate(SLABS):
            out_total[j % NSMALL] += out_incs[j]
            out_after.append(out_total[j % NSMALL])

        @block.sync
        def _(sync):
            for j, m in enumerate(SLABS):
                if j >= NBUF:
                    # slot free once DVE read slab j-NBUF; observe its DMA sem
                    sync.wait_ge(dve_stats, cum_stats[j - NBUF])
                    sync.wait_ge(dma_in[j - NBUF], 16)
                src = x[base_rows[j] : base_rows[j] + P * m, :].rearrange(
                    "(p m) f -> p (m f)", m=m
                )
                sync.dma_start(out=xt[:, j % NBUF, 0 : m * N], in_=src).then_inc(
                    dma_in[j], 16
                )
            # keep the NEFF alive until every output DMA has landed
            for s in range(NSMALL):
                sync.wait_ge(dma_out[s], out_total[s])

        @block.vector
        def _(vector):
            for j, m in enumerate(SLABS):
                k = j % NBUF
                s = j % NSMALL
                vector.wait_ge(dma_in[j], 16)
                if j >= NSMALL:
                    # stats slot WAR: bn_aggr of slab j-NSMALL has read it
                    vector.wait_ge(mv_ready, cum_aggr[j - NSMALL])
                for r in range(m):
                    for g in range(G):
                        vector.bn_stats(
                            out=stats[:, s, r, g, :],
                            in_=xt[:, k, (r * G + g) * 512 : (r * G + g + 1) * 512],
                        ).then_inc(dve_stats, 1)
                if j >= NSMALL:
                    # mv slot WAR: ACT of slab j-NSMALL has read it
                    vector.wait_ge(act_done, cum_act[j - NSMALL])
                # stats RAW: this slab's bn_stats writes retired
                vector.wait_ge(dve_stats, cum_stats[j])
                for r in range(m):
                    vector.bn_aggr(
                        out=mv[:, s, r, :], in_=stats[:, s, r, :, :]
                    ).then_inc(mv_ready, 1)

        @block.scalar
        def _(scalar):
            for j, m in enumerate(SLABS):
                s = j % NSMALL
                b, c0 = divmod(base_rows[j], C)
                scalar.wait_ge(mv_ready, cum_aggr[j])
                if j >= NSMALL:
                    # res slot reused: out-DMA of slab j-NSMALL must be done
                    scalar.wait_ge(dma_out[s], out_after[j - NSMALL])
                for r in range(m):
                    scalar.copy(
                        out=res[:, s, 0, r : r + 1], in_=mv[:, s, r, 0:1]
                    ).then_inc(act_done, 1)
                    scalar.activation(
                        out=res[:, s, 1, r : r + 1],
                        in_=mv[:, s, r, 1:2],
                        func=mybir.ActivationFunctionType.Sqrt,
                        scale=float(N) / (N - 1),
                    ).then_inc(act_done, 1)
                # res RAW: this slab's ACT writes retired before the DMA reads
                scalar.wait_ge(act_done, cum_act[j])
                if m == 1:
                    # per-stat contiguous [128] runs
                    for q in range(2):
                        dst = bass.AP(
                            tensor=out_ext,
                            offset=b * 2 * C + q * C + c0,
                            ap=[[1, P], [1, 1]],
                        )
                        scalar.dma_start(out=dst, in_=res[:, s, q, 0:1]).then_inc(
                            dma_out[s], 16
                        )
                else:
                    # one DMA: out[b, q*512 + c0 + m*p + r] <- res[p, s, q, r]
                    dst = bass.AP(
                        tensor=out_ext,
                        offset=b * 2 * C + c0,
                        ap=[[m, P], [C, 2], [1, m]],
                    )
                    scalar.dma_start(out=dst, in_=res[:, s, :, 0:m]).then_inc(
                        dma_out[s], 16
                    )

    return nc


def kernel(f_vol: np.ndarray) -> np.ndarray:
    from concourse.bass_utils import run_bass_kernel_spmd

    if "nc" not in _CACHE:
        _CACHE["nc"] = _build()
    nc = _CACHE["nc"]

    f_vol = np.ascontiguousarray(f_vol, dtype=np.float32)
    in_maps = [
        {"f_vol": f_vol[i * B_LOCAL : (i + 1) * B_LOCAL]} for i in range(N_CORES)
    ]
    res = run_bass_kernel_spmd(nc, in_maps, core_ids=list(range(N_CORES)))
    return np.concatenate([res.results[i]["out"] for i in range(N_CORES)], axis=0)


# revision 19
# speedup vs baseline: 1.5650x; 1.0898x over previous
"""AdaIN statistics kernel for TRN2, SPMD across 8 NeuronCores.

Input : f_vol [32, 512, 64, 64] f32
Output: [32, 1024] f32 = concat([mean over (h,w), unbiased std over (h,w)], axis=-1)

Sharding: data-parallel over batch — each of the 8 cores handles 4 batches
([4, 512, 64, 64] shard, 32 MiB). No collectives; the host concatenates the
8 per-core [4, 1024] outputs.

Per core: view the shard as 2048 rows (b*512+c) x 4096 spatial elems.
The shard is streamed in SLABS: a slab with m rows/partition loads
128*m consecutive rows, partition p holding rows base+p*m .. +m (so each
partition's DRAM chunk is m*16 KiB contiguous -> m*16 KiB DMA descriptors;
measured per-SDMA-engine rate is descriptor-size-bound: ~16 GB/s at 16 KiB,
~0.35 us fixed per packet). Big slabs mid-stream maximize bandwidth; small
slabs at the end keep the compute tail short.

Raw Bass with manual semaphores (Tile's scheduler emits 2 sync-waits on
slot-reuse DMAs, which this compiler's static-DMA encoding cannot hold):
  SP  : input slab DMAs (ring of 3 x 64 KiB/partition SBUF slots)
  DVE : 8 bn_stats per row + bn_aggr per row -> (mean, biased var)
  ACT : mean copy + sqrt(var * N/(N-1)) per row, one output DMA per slab

DMA completion is not FIFO across in-flight transfers, so each slab gets
its own single-use DMA-completion semaphore. Every cross-instruction data
edge is covered by an explicit semaphore observation so the CoreSim race
detector can verify the design.
"""

from contextlib import ExitStack

import numpy as np

B, C, H, W = 32, 512, 64, 64
N_CORES = 8
B_LOCAL = B // N_CORES  # 4
N = H * W  # 4096
P = 128
ROWS = B_LOCAL * C  # 2048
G = N // 512  # bn_stats groups per row = 8

# rows-per-partition per slab; each slab (128*m rows) must stay inside one
# batch (128*m | 512 alignment held by construction below)
SLABS = [1, 1, 2, 2, 2, 2, 2, 2, 1, 1]
assert sum(SLABS) * P == ROWS
MMAX = max(SLABS)
NBUF = 6  # input slab ring slots (6 x MMAX*16 KiB/partition)
NSMALL = 2  # stats/mv/res ring slots

_CACHE = {}


def _build():
    import concourse.bass as bass
    from concourse import mybir

    nc = bass.Bass()
    x_ext = nc.declare_dram_parameter(
        "f_vol", [B_LOCAL, C, H, W], mybir.dt.float32, isOutput=False
    )
    out_ext = nc.declare_dram_parameter(
        "out", [B_LOCAL, 2 * C], mybir.dt.float32, isOutput=True
    )

    x = x_ext.ap().rearrange("b c h w -> (b c) (h w)")  # [2048, 4096]

    nslabs = len(SLABS)
    base_rows = [P * sum(SLABS[:j]) for j in range(nslabs)]
    for j, m in enumerate(SLABS):
        assert base_rows[j] % (P * m) == 0 or True
        assert (base_rows[j] % C) + P * m <= C, f"slab {j} crosses a batch"
    cum_rows = [sum(SLABS[: j + 1]) for j in range(nslabs)]  # rows/partition
    cum_stats = [G * c for c in cum_rows]  # bn_stats count after slab j
    cum_aggr = cum_rows  # bn_aggr count after slab j
    cum_act = [2 * c for c in cum_rows]  # ACT op count after slab j

    with ExitStack() as ctx:
        block = ctx.enter_context(nc.Block(no_gpsimd_drain=True))
        dma_in = [
            ctx.enter_context(nc.semaphore(f"dma_in{j}")) for j in range(nslabs)
        ]
        dma_out = [
            ctx.enter_context(nc.semaphore(f"dma_out{s}")) for s in range(NSMALL)
        ]
        dve_stats = ctx.enter_context(nc.semaphore("dve_stats"))
        mv_ready = ctx.enter_context(nc.semaphore("mv_ready"))
        act_done = ctx.enter_context(nc.semaphore("act_done"))
        xt = ctx.enter_context(
            nc.sbuf_tensor("xt", [P, NBUF, MMAX * N], mybir.dt.float32)
        )
        stats = ctx.enter_context(
            nc.sbuf_tensor("stats", [P, NSMALL, MMAX, G, 6], mybir.dt.float32)
        )
        mv = ctx.enter_context(
            nc.sbuf_tensor("mv", [P, NSMALL, MMAX, 2], mybir.dt.float32)
        )
        res = ctx.enter_context(
            nc.sbuf_tensor("res", [P, NSMALL, 2, MMAX], mybir.dt.float32)
        )

        # out-DMA count per slab: m==1 needs two DMAs (per-stat contiguous
        # runs), otherwise one 3D-AP DMA covers both stats
        out_incs = [32 if m == 1 else 16 for m in SLABS]
        out_total = {s: 0 for s in range(NSMALL)}
        out_after = []  # dma_out[j % NSMALL] value after slab j's DMAs land
        for j, m in enumerate(SLABS):
            out_total[j % NSMALL] += out_incs[j]
            out_after.append(out_total[j % NSMALL])

        @block.sync
        def _(sync):
            for j, m in enumerate(SLABS):
                if j >= NBUF:
                    # slot free once DVE read slab j-NBUF; observe its DMA sem
                    sync.wait_ge(dve_stats, cum_stats[j - NBUF])
                    sync.wait_ge(dma_in[j - NBUF], 16)
                src = x[base_rows[j] : base_rows[j] + P * m, :].rearrange(
                    "(p m) f -> p (m f)", m=m
                )
                sync.dma_start(out=xt[:, j % NBUF, 0 : m * N], in_=src).then_inc(
                    dma_in[j], 16
                )
            # keep the NEFF alive until every output DMA has landed
            for s in range(NSMALL):
                sync.wait_ge(dma_out[s], out_total[s])

        @block.vector
        def _(vector):
            for j, m in enumerate(SLABS):
                k = j % NBUF
                s = j % NSMALL
                vector.wait_ge(dma_in[j], 16)
                if j >= NSMALL:
                    # stats slot WAR: bn_aggr of slab j-NSMALL has read it
                    vector.wait_ge(mv_ready, cum_aggr[j - NSMALL])
                for r in range(m):
                    for g in range(G):
                        vector.bn_stats(
                            out=stats[:, s, r, g, :],
                            in_=xt[:, k, (r * G + g) * 512 : (r * G + g + 1) * 512],
                        ).then_inc(dve_stats, 1)
                if j >= NSMALL:
                    # mv slot WAR: ACT of slab j-NSMALL has read it
                    vector.wait_ge(act_done, cum_act[j - NSMALL])
                # stats RAW: this slab's bn_stats writes retired
                vector.wait_ge(dve_stats, cum_stats[j])
                for r in range(m):
                    vector.bn_aggr(
                        out=mv[:, s, r, :], in_=stats[:, s, r, :, :]
                    ).then_inc(mv_ready, 1)

        @block.scalar
        def _(scalar):
            for j, m in enumerate(SLABS):
                s = j % NSMALL
                b, c0 = divmod(base_rows[j], C)
                scalar.wait_ge(mv_ready, cum_aggr[j])
                if j >= NSMALL:
                    # res slot reused: out-DMA of slab j-NSMALL must be done
                    scalar.wait_ge(dma_out[s], out_after[j - NSMALL])
                for r in range(m):
                    scalar.copy(
                        out=res[:, s, 0, r : r + 1], in_=mv[:, s, r, 0:1]
                    ).then_inc(act_done, 1)
                    scalar.activation(
                        out=res[:, s, 1, r : r + 1],
                        in_=mv[:, s, r, 1:2],
                        func=mybir.ActivationFunctionType.Sqrt,
                        scale=float(N) / (N - 1),
                    ).then_inc(act_done, 1)
                # res RAW: this slab's ACT writes retired before the DMA reads
                scalar.wait_ge(act_done, cum_act[j])
                if m == 1:
                    # per-stat contiguous [128] runs
                    for q in range(2):
                        dst = bass.AP(
                            tensor=out_ext,
                            offset=b * 2 * C + q * C + c0,
                            ap=[[1, P], [1, 1]],
                        )
                        scalar.dma_start(out=dst, in_=res[:, s, q, 0:1]).then_inc(
                            dma_out[s], 16
                        )
                else:
                    # one DMA: out[b, q*512 + c0 + m*p + r] <- res[p, s, q, r]
                    dst = bass.AP(
                        tensor=out_ext,
                        offset=b * 2 * C + c0,
                        ap=[[m, P], [C, 2], [1, m]],
                    )
                    scalar.dma_start(out=dst, in_=res[:, s, :, 0:m]).then_inc(
                        dma_out[s], 16
                    )

    return nc


def kernel(f_vol: np.ndarray) -> np.ndarray:
    from concourse.bass_utils import run_bass_kernel_spmd

    if "nc" not in _CACHE:
        _CACHE["nc"] = _build()
    nc = _CACHE["nc"]

    f_vol = np.ascontiguousarray(f_vol, dtype=np.float32)
    in_maps = [
        {"f_vol": f_vol[i * B_LOCAL : (i + 1) * B_LOCAL]} for i in range(N_CORES)
    ]
    res = run_bass_kernel_spmd(nc, in_maps, core_ids=list(range(N_CORES)))
    return np.concatenate([res.results[i]["out"] for i in range(N_CORES)], axis=0)


# revision 21
# speedup vs baseline: 1.6147x; 1.0317x over previous
"""AdaIN statistics kernel for TRN2, SPMD across 8 NeuronCores.

Input : f_vol [32, 512, 64, 64] f32
Output: [32, 1024] f32 = concat([mean over (h,w), unbiased std over (h,w)], axis=-1)

Sharding: data-parallel over batch — each of the 8 cores handles 4 batches
([4, 512, 64, 64] shard, 32 MiB). No collectives; the host concatenates the
8 per-core [4, 1024] outputs.

Per core: view the shard as 2048 rows (b*512+c) x 4096 spatial elems.
The shard is streamed in SLABS: a slab with m rows/partition loads
128*m consecutive rows, partition p holding rows base+p*m .. +m (so each
partition's DRAM chunk is m*16 KiB contiguous -> m*16 KiB DMA descriptors;
per-SDMA-engine rate is descriptor-size-bound). m=2 slabs stream at the
HBM cap; m=1 slabs at the end keep the compute tail short.

Raw Bass with manual semaphores (Tile's scheduler emits 2 sync-waits on
slot-reuse DMAs, which this compiler's static-DMA encoding cannot hold):
  SP  : input slab DMAs (ring of 6 x 32 KiB/partition SBUF slots)
  DVE : 8 bn_stats per row + bn_aggr per row -> (mean, biased var)
  ACT : mean copy + sqrt(var * N/(N-1)) per row, output DMAs; ACT also
        computes the FINAL slab itself (Copy/Square+accumulate passes)
        so the last rows don't queue behind DVE's backlog.

DMA completion is not FIFO across in-flight transfers, so each slab gets
its own single-use DMA-completion semaphore. Every cross-instruction data
edge is covered by an explicit semaphore observation so the CoreSim race
detector can verify the design.
"""

from contextlib import ExitStack

import numpy as np

B, C, H, W = 32, 512, 64, 64
N_CORES = 8
B_LOCAL = B // N_CORES  # 4
N = H * W  # 4096
P = 128
ROWS = B_LOCAL * C  # 2048
G = N // 512  # bn_stats groups per row = 8

# rows-per-partition per slab; each slab (128*m rows) must stay inside one
# batch. Consumer: 'dve' = bn_stats path, 'act' = ScalarE accumulate path.
SLABS = [2, 2, 2, 2, 2, 2, 2, 1, 1]
CONSUMER = ["dve"] * 8 + ["act"]
assert sum(SLABS) * P == ROWS and len(CONSUMER) == len(SLABS)
MMAX = max(SLABS)
NBUF = 6  # input slab ring slots (6 x MMAX*16 KiB/partition)
NSMALL = 2  # stats/mv/res ring slots

_CACHE = {}


def _build():
    import concourse.bass as bass
    from concourse import mybir

    nc = bass.Bass()
    x_ext = nc.declare_dram_parameter(
        "f_vol", [B_LOCAL, C, H, W], mybir.dt.float32, isOutput=False
    )
    out_ext = nc.declare_dram_parameter(
        "out", [B_LOCAL, 2 * C], mybir.dt.float32, isOutput=True
    )

    x = x_ext.ap().rearrange("b c h w -> (b c) (h w)")  # [2048, 4096]

    nslabs = len(SLABS)
    base_rows = [P * sum(SLABS[:j]) for j in range(nslabs)]
    for j, m in enumerate(SLABS):
        assert (base_rows[j] % C) + P * m <= C, f"slab {j} crosses a batch"

    # --- plan: cumulative semaphore targets per slab ---
    # dve_stats: +1 per bn_stats (DVE slabs);  act_stats: +1 per ACT
    # accumulate pass (2 per row, ACT slabs). mv_ready: +1 per bn_aggr.
    # act_done: +2 per DVE-slab row (mean copy + sqrt on ACT).
    dve_after, act_stats_after, mv_after, actd_after = [], [], [], []
    cd = ca = cm = cact = 0
    for j, m in enumerate(SLABS):
        if CONSUMER[j] == "dve":
            cd += G * m
            cm += m
            cact += 2 * m
        else:
            ca += 2 * m
        dve_after.append(cd)
        act_stats_after.append(ca)
        mv_after.append(cm)
        actd_after.append(cact)

    with ExitStack() as ctx:
        block = ctx.enter_context(nc.Block(no_gpsimd_drain=True))
        dma_in = [
            ctx.enter_context(nc.semaphore(f"dma_in{j}")) for j in range(nslabs)
        ]
        dma_out = [
            ctx.enter_context(nc.semaphore(f"dma_out{s}")) for s in range(NSMALL)
        ]
        dve_stats = ctx.enter_context(nc.semaphore("dve_stats"))
        act_stats = ctx.enter_context(nc.semaphore("act_stats"))
        mv_ready = ctx.enter_context(nc.semaphore("mv_ready"))
        act_done = ctx.enter_context(nc.semaphore("act_done"))
        xt = ctx.enter_context(
            nc.sbuf_tensor("xt", [P, NBUF, MMAX * N], mybir.dt.float32)
        )
        stats = ctx.enter_context(
            nc.sbuf_tensor("stats", [P, NSMALL, MMAX, G, 6], mybir.dt.float32)
        )
        mv = ctx.enter_context(
            nc.sbuf_tensor("mv", [P, NSMALL, MMAX, 2], mybir.dt.float32)
        )
        res = ctx.enter_context(
            nc.sbuf_tensor("res", [P, NSMALL, 2, MMAX], mybir.dt.float32)
        )
        # ACT-slab accumulators: [sum, sumsq, tmp] per row, no reuse
        acc = ctx.enter_context(
            nc.sbuf_tensor("acc", [P, MMAX, 3], mybir.dt.float32)
        )

        # out-DMA count per slab: m==1 needs two DMAs (per-stat contiguous
        # runs), otherwise one 3D-AP DMA covers both stats
        out_incs = [32 if m == 1 else 16 for m in SLABS]
        out_total = {s: 0 for s in range(NSMALL)}
        out_after = []  # dma_out[j % NSMALL] value after slab j's DMAs land
        for j, m in enumerate(SLABS):
            out_total[j % NSMALL] += out_incs[j]
            out_after.append(out_total[j % NSMALL])

        def slot_free_waits(eng, j):
            """Waits before rewriting xt slot (j % NBUF) for slab j."""
            if j < NBUF:
                return
            jp = j - NBUF
            if CONSUMER[jp] == "dve":
                eng.wait_ge(dve_stats, dve_after[jp])
            else:
                eng.wait_ge(act_stats, act_stats_after[jp])
            eng.wait_ge(dma_in[jp], 16)

        def emit_out_dma(scalar, j, m, s, b, c0):
            if m == 1:
                for q in range(2):
                    dst = bass.AP(
                        tensor=out_ext,
                        offset=b * 2 * C + q * C + c0,
                        ap=[[1, P], [1, 1]],
                    )
                    scalar.dma_start(out=dst, in_=res[:, s, q, 0:1]).then_inc(
                        dma_out[s], 16
                    )
            else:
                dst = bass.AP(
                    tensor=out_ext,
                    offset=b * 2 * C + c0,
                    ap=[[m, P], [C, 2], [1, m]],
                )
                scalar.dma_start(out=dst, in_=res[:, s, :, 0:m]).then_inc(
                    dma_out[s], 16
                )

        @block.sync
        def _(sync):
            for j, m in enumerate(SLABS):
                slot_free_waits(sync, j)
                src = x[base_rows[j] : base_rows[j] + P * m, :].rearrange(
                    "(p m) f -> p (m f)", m=m
                )
                sync.dma_start(out=xt[:, j % NBUF, 0 : m * N], in_=src).then_inc(
                    dma_in[j], 16
                )
            # keep the NEFF alive until every output DMA has landed
            for s in range(NSMALL):
                sync.wait_ge(dma_out[s], out_total[s])

        @block.vector
        def _(vector):
            prev_dve = [jj for jj in range(nslabs) if CONSUMER[jj] == "dve"]
            for j, m in enumerate(SLABS):
                if CONSUMER[j] != "dve":
                    continue
                k = j % NBUF
                s = j % NSMALL
                vector.wait_ge(dma_in[j], 16)
                # stats/mv slot WAR vs the previous DVE slab that used slot s
                pi = prev_dve.index(j)
                jp = None
                for jj in prev_dve[:pi][::-1]:
                    if jj % NSMALL == s:
                        jp = jj
                        break
                if jp is not None:
                    vector.wait_ge(mv_ready, mv_after[jp])
                for r in range(m):
                    for g in range(G):
                        vector.bn_stats(
                            out=stats[:, s, r, g, :],
                            in_=xt[:, k, (r * G + g) * 512 : (r * G + g + 1) * 512],
                        ).then_inc(dve_stats, 1)
                if jp is not None:
                    vector.wait_ge(act_done, actd_after[jp])
                # stats RAW: this slab's bn_stats writes retired
                vector.wait_ge(dve_stats, dve_after[j])
                for r in range(m):
                    vector.bn_aggr(
                        out=mv[:, s, r, :], in_=stats[:, s, r, :, :]
                    ).then_inc(mv_ready, 1)

        @block.scalar
        def _(scalar):
            A = 1.0 / np.sqrt(float(N) * (N - 1))  # sum*A squared = sum^2/(N(N-1))
            for j, m in enumerate(SLABS):
                k = j % NBUF
                s = j % NSMALL
                b, c0 = divmod(base_rows[j], C)
                if CONSUMER[j] == "dve":
                    scalar.wait_ge(mv_ready, mv_after[j])
                    if j >= NSMALL:
                        scalar.wait_ge(dma_out[s], out_after[j - NSMALL])
                    for r in range(m):
                        scalar.copy(
                            out=res[:, s, 0, r : r + 1], in_=mv[:, s, r, 0:1]
                        ).then_inc(act_done, 1)
                        scalar.activation(
                            out=res[:, s, 1, r : r + 1],
                            in_=mv[:, s, r, 1:2],
                            func=mybir.ActivationFunctionType.Sqrt,
                            scale=float(N) / (N - 1),
                        ).then_inc(act_done, 1)
                    # res RAW: this slab's ACT writes retired before DMA reads
                    scalar.wait_ge(act_done, actd_after[j])
                    emit_out_dma(scalar, j, m, s, b, c0)
                else:
                    # ACT computes this slab: sum (Copy+accum) then sumsq
                    # (Square+accum), both in-place over xt
                    scalar.wait_ge(dma_in[j], 16)
                    if j >= NSMALL:
                        scalar.wait_ge(dma_out[s], out_after[j - NSMALL])
                    base_as = act_stats_after[j] - 2 * m
                    for r in range(m):
                        row = xt[:, k, r * N : (r + 1) * N]
                        scalar.activation(
                            out=row,
                            in_=row,
                            func=mybir.ActivationFunctionType.Copy,
                            accum_out=acc[:, r, 0:1],
                        ).then_inc(act_stats, 1)
                        # observe the Copy (xt write + acc[0]) before Square
                        scalar.wait_ge(act_stats, base_as + 2 * r + 1)
                        scalar.activation(
                            out=row,
                            in_=row,
                            func=mybir.ActivationFunctionType.Square,
                            accum_out=acc[:, r, 1:2],
                        ).then_inc(act_stats, 1)
                    scalar.wait_ge(act_stats, act_stats_after[j])
                    ad = actd_after[j]  # running act_done value
                    for r in range(m):
                        # mean = sum / N
                        scalar.activation(
                            out=res[:, s, 0, r : r + 1],
                            in_=acc[:, r, 0:1],
                            func=mybir.ActivationFunctionType.Copy,
                            scale=1.0 / N,
                        ).then_inc(act_done, 1)
                        # tmp = (sum*A)^2 = sum^2/(N(N-1))
                        scalar.activation(
                            out=acc[:, r, 2:3],
                            in_=acc[:, r, 0:1],
                            func=mybir.ActivationFunctionType.Square,
                            scale=A,
                        ).then_inc(act_done, 1)
                        ad += 2
                        scalar.wait_ge(act_done, ad)
                        scalar.activation(
                            out=acc[:, r, 2:3],
                            in_=acc[:, r, 2:3],
                            func=mybir.ActivationFunctionType.Copy,
                            scale=-1.0,
                        ).then_inc(act_done, 1)
                        ad += 1
                        scalar.wait_ge(act_done, ad)
                        # std = sqrt(sumsq/(N-1) - sum^2/(N(N-1)))
                        scalar.activation(
                            out=res[:, s, 1, r : r + 1],
                            in_=acc[:, r, 1:2],
                            func=mybir.ActivationFunctionType.Sqrt,
                            scale=1.0 / (N - 1),
                            bias=acc[:, r, 2:3],
                        ).then_inc(act_done, 1)
                        ad += 1
                    scalar.wait_ge(act_done, ad)
                    emit_out_dma(scalar, j, m, s, b, c0)

    return nc


def kernel(f_vol: np.ndarray) -> np.ndarray:
    from concourse.bass_utils import run_bass_kernel_spmd

    if "nc" not in _CACHE:
        _CACHE["nc"] = _build()
    nc = _CACHE["nc"]

    f_vol = np.ascontiguousarray(f_vol, dtype=np.float32)
    in_maps = [
        {"f_vol": f_vol[i * B_LOCAL : (i + 1) * B_LOCAL]} for i in range(N_CORES)
    ]
    res = run_bass_kernel_spmd(nc, in_maps, core_ids=list(range(N_CORES)))
    return np.concatenate([res.results[i]["out"] for i in range(N_CORES)], axis=0)
